# revision 2
# baseline (speedup 1.0000x reference)
"""Trainium2 Bass kernel for nn_Luong_61684320305412 (bidirectional masked
softmax attention, B=8, L0=L1=2048, D=256).

Sharding: data-parallel over batch B across the 8 NeuronCores. Per core:

    S    = q0 @ q1^T                  [fp8e4 DoubleRow matmuls, K=256/instr]
    E    = exp(S/256), then masked entries forced to exactly 0 on DVE via
           one fused select per stripe: E = min(HUGE(1-m1[j]) + HUGE(1-m0[p]), E)
    E^T  = DMA xbar transposes of E stripes (fp16) — zero PE/Act cost
    out0 = (E^T-chains) (E   @ [q1 | 16])[:, 0:256] / col 256
    out1 = (E^T @ [q0 | 16])[:, 0:256] / col 256    (fp16 chains, raw q)

Key performance notes (vs the previous PE-transpose version at ~150 us):
  - E^T comes from DMA transpose-mode (xbar) SBUF->SBUF: 14 ns per 16x128
    tile => ~1.8 us per [128,2048] stripe on an (idle) DMA queue. This
    removes ~26 us of PE transposes and ~38 us of Act PSUM drains.
  - q^T (the fp8 DoubleRow S layout) is also built with DMA transposes of
    the fp16 q tiles + one fused cast per 4-tile group.
  - Raw q (not q/16) is kept in the fp16 rhs; the softmax 1/16 scale is
    folded into the sums column (value 16.0) so the final per-row scale
    (1/(16*sum)) comes from the existing reciprocal for free.
  - Engines end up: PE ~80us (S fp8-DR + 512 fp16 out-matmuls, the
    roofline for this algorithm), Act ~exp + drains, DVE ~mask + casts,
    with E^T traffic on DMA. Out-writes dispatch from the GpSimd SW-DGE
    so they never block the sync queue's transpose dispatches.
"""

import math
from contextlib import ExitStack

import numpy as np

import concourse.bass as bass
import concourse.tile as tile
from concourse import bacc, mybir
from concourse.bass_utils import run_bass_kernel_spmd

P = 128
B = 8
L = 2048          # L0 == L1
D = 256
T = L // P        # 16 row tiles
AUGW = 272        # 256 data | 2 cols of 16.0 | 14 junk (272*2B = 32B-aligned tiles)
NSUM = D + 2      # matmul rhs width: data + sum cols
HUGE = 60000.0    # fp16-exact; mask select: min(HUGE*(1-m1[j]) + HUGE*(1-m0[p]), E)
SCALE2 = 1.0 / 256.0   # applied to raw scores inside exp
SUMC = 16.0       # sums column value; final scale = 1/(16*sum)

f32 = mybir.dt.float32
f16 = mybir.dt.float16
f8 = mybir.dt.float8e4
i32 = mybir.dt.int32
MUL = mybir.AluOpType.mult
EXP = mybir.ActivationFunctionType.Exp
DR = mybir.MatmulPerfMode.DoubleRow


def _emit(tc: tile.TileContext, ctx: ExitStack, io: dict):
    nc = tc.nc
    q0, q1, m0, m1 = io["q0"], io["q1"], io["mask0"], io["mask1"]
    out0, out1 = io["out0"], io["out1"]

    consts = ctx.enter_context(tc.tile_pool(name="consts", bufs=1))
    stage = ctx.enter_context(tc.tile_pool(name="stage", bufs=2))
    stg_t = ctx.enter_context(tc.tile_pool(name="stg_t", bufs=2))
    qpool = ctx.enter_context(tc.tile_pool(name="qpool", bufs=1))
    e_pool = ctx.enter_context(tc.tile_pool(name="e", bufs=1))
    outp = ctx.enter_context(tc.tile_pool(name="outp", bufs=2))
    small = ctx.enter_context(tc.tile_pool(name="small", bufs=4))
    s_psum = ctx.enter_context(tc.tile_pool(name="s_psum", bufs=2, space="PSUM"))
    o_psum = ctx.enter_context(tc.tile_pool(name="o_psum", bufs=4, space="PSUM"))

    # ---- persistent operand tiles ----
    q0a = qpool.tile([P, T, AUGW], f16)   # raw q fp16 | 16.0 cols (out-matmul rhs)
    q1a = qpool.tile([P, T, AUGW], f16)
    q0t = qpool.tile([P, 2, L], f8)       # raw q, [d%128, d//128, l] DR layout
    q1t = qpool.tile([P, 2, L], f8)
    e0 = e_pool.tile([P, T, L], f16)      # E  [l0, l1]
    e1 = e_pool.tile([P, T, L], f16)      # E^T [l1, l0] (built by DMA transpose)

    nc.vector.memset(q0a[:, :, D:NSUM], SUMC)
    nc.vector.memset(q1a[:, :, D:NSUM], SUMC)

    # ---- mask prep ----
    # row tile: wm1[p, j] = HUGE*(1-m1[j]) for all p (PE outer-product bcast);
    # col tile: s0[p, t] = HUGE*(1-m0[t*128+p])
    m1i = consts.tile([1, L], i32)
    nc.sync.dma_start(out=m1i, in_=m1.rearrange("(o l) -> o l", o=1))
    m1f = consts.tile([1, L], f32)
    nc.vector.tensor_copy(out=m1f, in_=m1i)
    wm1row = consts.tile([1, L], f16)
    nc.vector.tensor_scalar(out=wm1row, in0=m1f, scalar1=-HUGE, scalar2=HUGE,
                            op0=MUL, op1=mybir.AluOpType.add)
    onesrow = consts.tile([1, P], f16)
    nc.vector.memset(onesrow, 1.0)
    wm1 = consts.tile([P, L], f16)

    m0i = consts.tile([P, T], i32)
    nc.sync.dma_start(out=m0i, in_=m0.rearrange("(t p) -> p t", p=P))
    m0fc = consts.tile([P, T], f32)
    nc.vector.tensor_copy(out=m0fc, in_=m0i)
    s0 = consts.tile([P, T], f32)
    nc.vector.tensor_scalar(out=s0, in0=m0fc, scalar1=-HUGE, scalar2=HUGE,
                            op0=MUL, op1=mybir.AluOpType.add)

    # broadcast wm1row -> wm1 via PE outer product (one-time)
    for c in range(4):
        pw = s_psum.tile([P, 512], f32, tag="sp", name=f"pw{c}")
        nc.tensor.matmul(pw, lhsT=onesrow, rhs=wm1row[:, c * 512:(c + 1) * 512],
                         start=True, stop=True)
        nc.scalar.copy(wm1[:, c * 512:(c + 1) * 512], pw)

    # ---- q prep: load group of 4 tiles, cast to f16, DMA-transpose each
    # tile into the group staging tile, one fused cast into the fp8 DR layout
    def prep(src, aug, tr, g, f8_on_act):
        st = stage.tile([P, 4, D], f32, tag="st")
        nc.sync.dma_start(
            out=st, in_=src.rearrange("(g t p) d -> g p t d", p=P, t=4)[g]
        )
        nc.vector.tensor_copy(out=aug[:, 4 * g:4 * g + 4, 0:D], in_=st)
        sg = stg_t.tile([P, 4, 2, P], f16, tag="sg")
        for tt in range(4):
            nc.sync.dma_start(
                out=sg[:, tt], in_=aug[:, 4 * g + tt, 0:D], transpose=True
            )
        dst = tr[:, :, g * 512:(g + 1) * 512]
        dstv = dst.rearrange("p c (tt f) -> p tt c f", tt=4)
        if f8_on_act:
            nc.scalar.copy(dstv, sg)
        else:
            nc.vector.tensor_copy(out=dstv, in_=sg)

    # ---- S matmuls (fp8 DR) + exp + fused mask-select ----
    def s_half(t, H):
        ps = s_psum.tile([P, 1024], f32, tag="sp")
        for c in range(2):
            off = H * 1024 + c * 512
            nc.tensor.matmul(
                ps[:, c * 512:(c + 1) * 512],
                lhsT=q0t[:, :, t * P:(t + 1) * P],
                rhs=q1t[:, :, off:off + 512],
                start=True, stop=True, perf_mode=DR,
            )
        sl = slice(H * 1024, (H + 1) * 1024)
        nc.scalar.activation(out=e0[:, t, sl], in_=ps, func=EXP, scale=SCALE2)
        # masked entries -> exactly 0: e0 = min(wm1 + s0[p], e0)
        nc.vector.scalar_tensor_tensor(
            out=e0[:, t, sl], in0=wm1[:, sl], scalar=s0[:, t:t + 1],
            in1=e0[:, t, sl],
            op0=mybir.AluOpType.add, op1=mybir.AluOpType.min,
        )

    # ---- E^T stripe via DMA xbar transpose ----
    def etr(t):
        nc.sync.dma_start(
            out=e1[:, :, t * P:(t + 1) * P], in_=e0[:, t, :], transpose=True
        )

    # ---- one pairwise-interleaved pair of out accumulation chains ----
    def out_pair(esrc, raug, odram, j0):
        pos = [o_psum.tile([P, NSUM], f32, tag="op", name=f"op{_k}") for _k in range(2)]
        for t in range(T):
            for k in range(2):
                j = j0 + k
                nc.tensor.matmul(
                    pos[k],
                    lhsT=esrc[:, t, j * P:(j + 1) * P],
                    rhs=raug[:, t, 0:NSUM],
                    start=(t == 0), stop=(t == T - 1),
                )
        ot = outp.tile([P, 2, D], f32, tag="ot")
        for k in range(2):
            rc = small.tile([P, 1], f32, tag="rc")
            nc.vector.reciprocal(rc, pos[k][:, D:D + 1])
            nc.scalar.mul(ot[:, k], pos[k][:, 0:D], rc)
        nc.gpsimd.dma_start(
            out=odram.rearrange("(j p) d -> p j d", p=P)[:, j0:j0 + 2, :], in_=ot
        )

    # ---- emission schedule ----
    prep(q1, q1a, q1t, 0, f8_on_act=True)
    prep(q1, q1a, q1t, 1, f8_on_act=True)
    prep(q0, q0a, q0t, 0, f8_on_act=False)
    for t in range(4):
        s_half(t, 0)
    prep(q1, q1a, q1t, 2, f8_on_act=True)
    prep(q1, q1a, q1t, 3, f8_on_act=True)
    prep(q0, q0a, q0t, 1, f8_on_act=False)
    for t in range(4, 8):
        s_half(t, 0)
    prep(q0, q0a, q0t, 2, f8_on_act=False)
    for t in range(8, 12):
        s_half(t, 0)
    prep(q0, q0a, q0t, 3, f8_on_act=False)
    for t in range(12, T):
        s_half(t, 0)
    for t in range(T):
        s_half(t, 1)
        etr(t)
        if t % 4 == 3:
            out_pair(e0, q0a, out1, (t // 4) * 2)
    out_pair(e0, q0a, out1, 8)
    out_pair(e1, q1a, out0, 0)
    out_pair(e0, q0a, out1, 10)
    out_pair(e1, q1a, out0, 2)
    out_pair(e0, q0a, out1, 12)
    out_pair(e1, q1a, out0, 4)
    out_pair(e0, q0a, out1, 14)
    out_pair(e1, q1a, out0, 6)
    for j0 in range(8, T, 2):
        out_pair(e1, q1a, out0, j0)


_CACHED_NC = None


def _build():
    global _CACHED_NC
    if _CACHED_NC is not None:
        return _CACHED_NC
    nc = bacc.Bacc("TRN2", target_bir_lowering=False, debug=False)
    io = {
        "q0": nc.dram_tensor("q0", [L, D], f32, kind="ExternalInput").ap(),
        "q1": nc.dram_tensor("q1", [L, D], f32, kind="ExternalInput").ap(),
        "mask0": nc.dram_tensor("mask0", [L], i32, kind="ExternalInput").ap(),
        "mask1": nc.dram_tensor("mask1", [L], i32, kind="ExternalInput").ap(),
        "out0": nc.dram_tensor("out0", [L, D], f32, kind="ExternalOutput").ap(),
        "out1": nc.dram_tensor("out1", [L, D], f32, kind="ExternalOutput").ap(),
    }
    with tile.TileContext(nc) as tc:
        with ExitStack() as ctx:
            _emit(tc, ctx, io)
    nc.compile()
    _CACHED_NC = nc
    return nc


def run_on_cores(q0, q1, mask0, mask1, trace=False):
    """Run the SPMD kernel; returns (out0, out1, BassKernelResults)."""
    nc = _build()
    in_maps = [
        {
            "q0": np.ascontiguousarray(q0[b], dtype=np.float32),
            "q1": np.ascontiguousarray(q1[b], dtype=np.float32),
            "mask0": np.ascontiguousarray(mask0[b], dtype=np.int32),
            "mask1": np.ascontiguousarray(mask1[b], dtype=np.int32),
        }
        for b in range(B)
    ]
    br = run_bass_kernel_spmd(nc, in_maps, list(range(B)), trace=trace)
    out0 = np.stack([br.results[b]["out0"] for b in range(B)])
    out1 = np.stack([br.results[b]["out1"] for b in range(B)])
    return out0, out1, br


def kernel(q0, q1, len0=None, len1=None, mask0=None, mask1=None, **_):
    q0 = np.asarray(q0, dtype=np.float32)
    q1 = np.asarray(q1, dtype=np.float32)
    mask0 = np.asarray(mask0, dtype=np.int32)
    mask1 = np.asarray(mask1, dtype=np.int32)
    out0, out1, _br = run_on_cores(q0, q1, mask0, mask1, trace=False)
    return out0, out1


# revision 3
# speedup vs baseline: 1.2538x; 1.2538x over previous
"""Trainium2 Bass kernel for nn_Luong_61684320305412 (bidirectional masked
softmax attention, B=8, L0=L1=2048, D=256).

Sharding: data-parallel over batch B across the 8 NeuronCores. Per core:

    S    = q0 @ q1^T                  [fp8e4 DoubleRow matmuls, K=256/instr]
    E    = exp(S/256), then masked entries forced to exactly 0 on DVE via
           one fused select per stripe: E = min(HUGE(1-m1[j]) + HUGE(1-m0[p]), E)
    E^T  = DMA xbar transpose of each masked E stripe (fp16, SBUF->SBUF)
    out0 = (E^T-chains @ [q1 | 16])[:, 0:256] / col 256   (fp16 chains, raw q)
    out1 = (E  -chains @ [q0 | 16])[:, 0:256] / col 256

Key performance notes (vs the 150 us PE-transpose version):
  - E^T comes from 16 DMA transpose-mode (xbar) instructions, one per
    [128,2048] stripe (~2.6 us each, serialized on the sync DGE but fully
    hidden behind compute). This removes ~26 us of PE transposes and
    ~30 us of Act PSUM drains. Descriptor-gen on the DGE is the per-tile
    cost, so only the 16 big stripe transposes go to the xbar; the q^T
    fp8-layout prep keeps cheap PE transposes.
  - Raw q (not q/16) is kept everywhere; the softmax 1/16 scale is folded
    into the sums column (value 16.0) so the final row scale 1/(16*sum)
    falls out of the existing reciprocal.
  - Out-writes dispatch from the GpSimd SW-DGE so the sync queue is free
    for the stripe transposes; q loads are emitted before any transpose.
  - fp8e4 DoubleRow for S; fp16 for the out chains (fp8 rhs would put
    ~3e-2 of quantization noise straight into the output).
"""

import math
from contextlib import ExitStack

import numpy as np

import concourse.bass as bass
import concourse.tile as tile
from concourse import bacc, mybir
from concourse.bass_utils import run_bass_kernel_spmd

P = 128
B = 8
L = 2048          # L0 == L1
D = 256
T = L // P        # 16 row tiles
AUGW = 272        # 256 data | 2 cols of 16.0 | junk (keeps 32B-aligned stripes)
NSUM = D + 2      # matmul rhs width: data + sum cols
HUGE = 60000.0    # fp16-exact; mask select: min(HUGE*(1-m1[j]) + HUGE*(1-m0[p]), E)
SCALE2 = 1.0 / 256.0   # applied to raw scores inside exp
SUMC = 16.0       # sums column value; final scale = 1/(16*sum)

f32 = mybir.dt.float32
f16 = mybir.dt.float16
f8 = mybir.dt.float8e4
i32 = mybir.dt.int32
MUL = mybir.AluOpType.mult
EXP = mybir.ActivationFunctionType.Exp
DR = mybir.MatmulPerfMode.DoubleRow


def _emit(tc: tile.TileContext, ctx: ExitStack, io: dict):
    nc = tc.nc
    q0, q1, m0, m1 = io["q0"], io["q1"], io["mask0"], io["mask1"]
    out0, out1 = io["out0"], io["out1"]

    consts = ctx.enter_context(tc.tile_pool(name="consts", bufs=1))
    stage = ctx.enter_context(tc.tile_pool(name="stage", bufs=4))
    qpool = ctx.enter_context(tc.tile_pool(name="qpool", bufs=1))
    e_pool = ctx.enter_context(tc.tile_pool(name="e", bufs=1))
    outp = ctx.enter_context(tc.tile_pool(name="outp", bufs=2))
    small = ctx.enter_context(tc.tile_pool(name="small", bufs=4))
    s_psum = ctx.enter_context(tc.tile_pool(name="s_psum", bufs=2, space="PSUM"))
    t_psum = ctx.enter_context(tc.tile_pool(name="t_psum", bufs=1, space="PSUM"))
    o_psum = ctx.enter_context(tc.tile_pool(name="o_psum", bufs=3, space="PSUM"))

    # ---- persistent operand tiles ----
    q0a = qpool.tile([P, T, AUGW], f16)   # raw q fp16 | 16.0 cols (out-matmul rhs)
    q1a = qpool.tile([P, T, AUGW], f16)
    q0t = qpool.tile([P, 2, L], f8)       # raw q, [d%128, d//128, l] DR layout
    q1t = qpool.tile([P, 2, L], f8)
    e0 = e_pool.tile([P, T, L], f16)      # E  [l0, l1]
    e1 = e_pool.tile([P, T, L], f16)      # E^T [l1, l0] (built by DMA xbar transpose)

    nc.vector.memset(q0a[:, :, D:NSUM], SUMC)
    nc.vector.memset(q1a[:, :, D:NSUM], SUMC)

    # ---- mask prep ----
    # row tile: wm1[p, j] = HUGE*(1-m1[j]) for all p (PE outer-product bcast);
    # col tile: s0[p, t] = HUGE*(1-m0[t*128+p])
    m1i = consts.tile([1, L], i32)
    nc.sync.dma_start(out=m1i, in_=m1.rearrange("(o l) -> o l", o=1))
    m1f = consts.tile([1, L], f32)
    nc.vector.tensor_copy(out=m1f, in_=m1i)
    wm1row = consts.tile([1, L], f16)
    nc.vector.tensor_scalar(out=wm1row, in0=m1f, scalar1=-HUGE, scalar2=HUGE,
                            op0=MUL, op1=mybir.AluOpType.add)
    onesrow = consts.tile([1, P], f16)
    nc.vector.memset(onesrow, 1.0)
    wm1 = consts.tile([P, L], f16)

    m0i = consts.tile([P, T], i32)
    nc.sync.dma_start(out=m0i, in_=m0.rearrange("(t p) -> p t", p=P))
    m0fc = consts.tile([P, T], f32)
    nc.vector.tensor_copy(out=m0fc, in_=m0i)
    s0 = consts.tile([P, T], f32)
    nc.vector.tensor_scalar(out=s0, in0=m0fc, scalar1=-HUGE, scalar2=HUGE,
                            op0=MUL, op1=mybir.AluOpType.add)

    from concourse.masks import make_identity
    ident_f = consts.tile([P, P], f32)
    make_identity(nc, ident_f)
    ident16 = consts.tile([P, P], f16)
    nc.vector.tensor_copy(out=ident16, in_=ident_f)

    # broadcast wm1row -> wm1 via PE outer product (one-time)
    for c in range(4):
        pw = s_psum.tile([P, 512], f32, tag="sp", name=f"pw{c}")
        nc.tensor.matmul(pw, lhsT=onesrow, rhs=wm1row[:, c * 512:(c + 1) * 512],
                         start=True, stop=True)
        nc.scalar.copy(wm1[:, c * 512:(c + 1) * 512], pw)

    # ---- load q, cast to f16 (raw), and PE-transpose into the fp8 DR layout ----
    def prep_pack(src, aug, tr, p4, on_act):
        pt = t_psum.tile([P, 1024], f16, tag="tp")
        for ti in range(4):
            t = p4 * 4 + ti
            st = stage.tile([P, D], f32, tag="st")
            nc.sync.dma_start(
                out=st, in_=src.rearrange("(t p) d -> t p d", p=P)[t]
            )
            if on_act:
                nc.scalar.copy(aug[:, t, 0:D], st)
            else:
                nc.vector.tensor_copy(out=aug[:, t, 0:D], in_=st)
            for dc in range(2):
                nc.tensor.transpose(
                    pt[:, (ti * 2 + dc) * P:(ti * 2 + dc + 1) * P],
                    aug[:, t, dc * P:(dc + 1) * P], ident16,
                )
        dst = tr[:, :, p4 * 512:(p4 + 1) * 512]
        dstv = dst.rearrange("p two (t f) -> p t two f", t=4)
        srcv = pt.rearrange("p (t two f) -> p t two f", t=4, two=2)
        if on_act:
            nc.scalar.copy(dstv, srcv)
        else:
            nc.vector.tensor_copy(out=dstv, in_=srcv)

    # ---- S matmuls (fp8 DR) + exp + fused mask-select ----
    def s_half(t, H):
        ps = s_psum.tile([P, 1024], f32, tag="sp")
        for c in range(2):
            off = H * 1024 + c * 512
            nc.tensor.matmul(
                ps[:, c * 512:(c + 1) * 512],
                lhsT=q0t[:, :, t * P:(t + 1) * P],
                rhs=q1t[:, :, off:off + 512],
                start=True, stop=True, perf_mode=DR,
            )
        sl = slice(H * 1024, (H + 1) * 1024)
        nc.scalar.activation(out=e0[:, t, sl], in_=ps, func=EXP, scale=SCALE2)
        # masked entries -> exactly 0: e0 = min(wm1 + s0[p], e0)
        nc.vector.scalar_tensor_tensor(
            out=e0[:, t, sl], in0=wm1[:, sl], scalar=s0[:, t:t + 1],
            in1=e0[:, t, sl],
            op0=mybir.AluOpType.add, op1=mybir.AluOpType.min,
        )

    # ---- E^T stripe via DMA xbar transpose (sync DGE) ----
    def etr(t):
        nc.sync.dma_start(
            out=e1[:, :, t * P:(t + 1) * P], in_=e0[:, t, :], transpose=True
        )

    # ---- one pairwise-interleaved pair of out accumulation chains ----
    def out_pair(esrc, raug, odram, j0):
        pos = [o_psum.tile([P, NSUM], f32, tag="op", name=f"op{_k}") for _k in range(2)]
        for t in range(T):
            for k in range(2):
                j = j0 + k
                nc.tensor.matmul(
                    pos[k],
                    lhsT=esrc[:, t, j * P:(j + 1) * P],
                    rhs=raug[:, t, 0:NSUM],
                    start=(t == 0), stop=(t == T - 1),
                )
        ot = outp.tile([P, 2, D], f32, tag="ot")
        for k in range(2):
            rc = small.tile([P, 1], f32, tag="rc")
            nc.vector.reciprocal(rc, pos[k][:, D:D + 1])
            nc.scalar.mul(ot[:, k], pos[k][:, 0:D], rc)
        nc.gpsimd.dma_start(
            out=odram.rearrange("(j p) d -> p j d", p=P)[:, j0:j0 + 2, :], in_=ot
        )

    # ---- emission schedule ----
    prep_pack(q1, q1a, q1t, 0, on_act=True)
    prep_pack(q1, q1a, q1t, 1, on_act=False)
    prep_pack(q0, q0a, q0t, 0, on_act=True)
    for t in range(4):
        s_half(t, 0)
    prep_pack(q1, q1a, q1t, 2, on_act=True)
    prep_pack(q1, q1a, q1t, 3, on_act=False)
    prep_pack(q0, q0a, q0t, 1, on_act=True)
    for t in range(4, 8):
        s_half(t, 0)
    prep_pack(q0, q0a, q0t, 2, on_act=False)
    for t in range(8, 12):
        s_half(t, 0)
    prep_pack(q0, q0a, q0t, 3, on_act=True)
    for t in range(12, T):
        s_half(t, 0)
    for t in range(T):
        s_half(t, 1)
        etr(t)
        if t % 4 == 3:
            out_pair(e0, q0a, out1, (t // 4) * 2)
    out_pair(e0, q0a, out1, 8)
    out_pair(e1, q1a, out0, 0)
    out_pair(e0, q0a, out1, 10)
    out_pair(e1, q1a, out0, 2)
    out_pair(e0, q0a, out1, 12)
    out_pair(e1, q1a, out0, 4)
    out_pair(e0, q0a, out1, 14)
    out_pair(e1, q1a, out0, 6)
    for j0 in range(8, T, 2):
        out_pair(e1, q1a, out0, j0)


_CACHED_NC = None


def _build():
    global _CACHED_NC
    if _CACHED_NC is not None:
        return _CACHED_NC
    nc = bacc.Bacc("TRN2", target_bir_lowering=False, debug=False)
    io = {
        "q0": nc.dram_tensor("q0", [L, D], f32, kind="ExternalInput").ap(),
        "q1": nc.dram_tensor("q1", [L, D], f32, kind="ExternalInput").ap(),
        "mask0": nc.dram_tensor("mask0", [L], i32, kind="ExternalInput").ap(),
        "mask1": nc.dram_tensor("mask1", [L], i32, kind="ExternalInput").ap(),
        "out0": nc.dram_tensor("out0", [L, D], f32, kind="ExternalOutput").ap(),
        "out1": nc.dram_tensor("out1", [L, D], f32, kind="ExternalOutput").ap(),
    }
    with tile.TileContext(nc) as tc:
        with ExitStack() as ctx:
            _emit(tc, ctx, io)
    nc.compile()
    _CACHED_NC = nc
    return nc


def run_on_cores(q0, q1, mask0, mask1, trace=False):
    """Run the SPMD kernel; returns (out0, out1, BassKernelResults)."""
    nc = _build()
    in_maps = [
        {
            "q0": np.ascontiguousarray(q0[b], dtype=np.float32),
            "q1": np.ascontiguousarray(q1[b], dtype=np.float32),
            "mask0": np.ascontiguousarray(mask0[b], dtype=np.int32),
            "mask1": np.ascontiguousarray(mask1[b], dtype=np.int32),
        }
        for b in range(B)
    ]
    br = run_bass_kernel_spmd(nc, in_maps, list(range(B)), trace=trace)
    out0 = np.stack([br.results[b]["out0"] for b in range(B)])
    out1 = np.stack([br.results[b]["out1"] for b in range(B)])
    return out0, out1, br


def kernel(q0, q1, len0=None, len1=None, mask0=None, mask1=None, **_):
    q0 = np.asarray(q0, dtype=np.float32)
    q1 = np.asarray(q1, dtype=np.float32)
    mask0 = np.asarray(mask0, dtype=np.int32)
    mask1 = np.asarray(mask1, dtype=np.int32)
    out0, out1, _br = run_on_cores(q0, q1, mask0, mask1, trace=False)
    return out0, out1


# revision 4
# speedup vs baseline: 1.2821x; 1.0225x over previous
"""Trainium2 Bass kernel for nn_Luong_61684320305412 (bidirectional masked
softmax attention, B=8, L0=L1=2048, D=256).

Sharding: data-parallel over batch B across the 8 NeuronCores. Per core:

    S    = q0 @ q1^T                  [fp8e4 DoubleRow matmuls, K=256/instr]
    E    = exp(S/256), then masked entries forced to exactly 0 on DVE via
           one fused select per stripe: E = min(HUGE(1-m1[j]) + HUGE(1-m0[p]), E)
    E^T  = DMA xbar transpose of each masked E stripe (fp16, SBUF->SBUF)
    out0 = (E^T-chains @ [q1 | 16])[:, 0:256] / col 256   (fp16 chains, raw q)
    out1 = (E  -chains @ [q0 | 16])[:, 0:256] / col 256

Key performance notes (vs the 150 us PE-transpose version):
  - E^T comes from 16 DMA transpose-mode (xbar) instructions, one per
    [128,2048] stripe (~2.6 us each, serialized on the sync DGE but fully
    hidden behind compute). This removes ~26 us of PE transposes and
    ~30 us of Act PSUM drains. Descriptor-gen on the DGE is the per-tile
    cost, so only the 16 big stripe transposes go to the xbar; the q^T
    fp8-layout prep keeps cheap PE transposes.
  - Raw q (not q/16) is kept everywhere; the softmax 1/16 scale is folded
    into the sums column (value 16.0) so the final row scale 1/(16*sum)
    falls out of the existing reciprocal.
  - Out-writes dispatch from the GpSimd SW-DGE so the sync queue is free
    for the stripe transposes; q loads are emitted before any transpose.
  - fp8e4 DoubleRow for S; fp16 for the out chains (fp8 rhs would put
    ~3e-2 of quantization noise straight into the output).
"""

import math
from contextlib import ExitStack

import numpy as np

import concourse.bass as bass
import concourse.tile as tile
from concourse import bacc, mybir
from concourse.bass_utils import run_bass_kernel_spmd

P = 128
B = 8
L = 2048          # L0 == L1
D = 256
T = L // P        # 16 row tiles
AUGW = 272        # 256 data | 2 cols of 16.0 | junk (keeps 32B-aligned stripes)
NSUM = D + 2      # matmul rhs width: data + sum cols
HUGE = 60000.0    # fp16-exact; mask select: min(HUGE*(1-m1[j]) + HUGE*(1-m0[p]), E)
SCALE2 = 1.0 / 256.0   # applied to raw scores inside exp
SUMC = 16.0       # sums column value; final scale = 1/(16*sum)

f32 = mybir.dt.float32
f16 = mybir.dt.float16
f8 = mybir.dt.float8e4
i32 = mybir.dt.int32
MUL = mybir.AluOpType.mult
EXP = mybir.ActivationFunctionType.Exp
DR = mybir.MatmulPerfMode.DoubleRow


def _emit(tc: tile.TileContext, ctx: ExitStack, io: dict):
    nc = tc.nc
    q0, q1, m0, m1 = io["q0"], io["q1"], io["mask0"], io["mask1"]
    out0, out1 = io["out0"], io["out1"]

    consts = ctx.enter_context(tc.tile_pool(name="consts", bufs=1))
    stage = ctx.enter_context(tc.tile_pool(name="stage", bufs=4))
    qpool = ctx.enter_context(tc.tile_pool(name="qpool", bufs=1))
    e_pool = ctx.enter_context(tc.tile_pool(name="e", bufs=1))
    outp = ctx.enter_context(tc.tile_pool(name="outp", bufs=2))
    small = ctx.enter_context(tc.tile_pool(name="small", bufs=4))
    s_psum = ctx.enter_context(tc.tile_pool(name="s_psum", bufs=2, space="PSUM"))
    t_psum = ctx.enter_context(tc.tile_pool(name="t_psum", bufs=1, space="PSUM"))
    o_psum = ctx.enter_context(tc.tile_pool(name="o_psum", bufs=3, space="PSUM"))

    # ---- persistent operand tiles ----
    q0a = qpool.tile([P, T, AUGW], f16)   # raw q fp16 | 16.0 cols (out-matmul rhs)
    q1a = qpool.tile([P, T, AUGW], f16)
    q0t = qpool.tile([P, 2, L], f8)       # raw q, [d%128, d//128, l] DR layout
    q1t = qpool.tile([P, 2, L], f8)
    e0 = e_pool.tile([P, T, L], f16)      # E  [l0, l1]
    e1 = e_pool.tile([P, T, L], f16)      # E^T [l1, l0] (built by DMA xbar transpose)

    nc.vector.memset(q0a[:, :, D:NSUM], SUMC)
    nc.vector.memset(q1a[:, :, D:NSUM], SUMC)

    # ---- mask prep ----
    # row tile: wm1[p, j] = HUGE*(1-m1[j]) for all p (PE outer-product bcast);
    # col tile: s0[p, t] = HUGE*(1-m0[t*128+p])
    m1i = consts.tile([1, L], i32)
    nc.sync.dma_start(out=m1i, in_=m1.rearrange("(o l) -> o l", o=1))
    m1f = consts.tile([1, L], f32)
    nc.vector.tensor_copy(out=m1f, in_=m1i)
    wm1row = consts.tile([1, L], f16)
    nc.vector.tensor_scalar(out=wm1row, in0=m1f, scalar1=-HUGE, scalar2=HUGE,
                            op0=MUL, op1=mybir.AluOpType.add)
    onesrow = consts.tile([1, P], f16)
    nc.vector.memset(onesrow, 1.0)
    wm1 = consts.tile([P, L], f16)

    m0i = consts.tile([P, T], i32)
    nc.sync.dma_start(out=m0i, in_=m0.rearrange("(t p) -> p t", p=P))
    m0fc = consts.tile([P, T], f32)
    nc.vector.tensor_copy(out=m0fc, in_=m0i)
    s0 = consts.tile([P, T], f32)
    nc.vector.tensor_scalar(out=s0, in0=m0fc, scalar1=-HUGE, scalar2=HUGE,
                            op0=MUL, op1=mybir.AluOpType.add)

    from concourse.masks import make_identity
    ident_f = consts.tile([P, P], f32)
    make_identity(nc, ident_f)
    ident16 = consts.tile([P, P], f16)
    nc.vector.tensor_copy(out=ident16, in_=ident_f)

    # broadcast wm1row -> wm1 via PE outer product (one-time)
    for c in range(4):
        pw = s_psum.tile([P, 512], f32, tag="sp", name=f"pw{c}")
        nc.tensor.matmul(pw, lhsT=onesrow, rhs=wm1row[:, c * 512:(c + 1) * 512],
                         start=True, stop=True)
        nc.scalar.copy(wm1[:, c * 512:(c + 1) * 512], pw)

    # ---- load q (one DMA per 4-tile group), cast to f16 (raw), and
    # PE-transpose into the fp8 DR layout ----
    def prep_pack(src, aug, tr, p4, on_act):
        pt = t_psum.tile([P, 1024], f16, tag="tp")
        st = stage.tile([P, 4, D], f32, tag="st")
        nc.sync.dma_start(
            out=st, in_=src.rearrange("(g t p) d -> g p t d", p=P, t=4)[p4]
        )
        dstc = aug[:, 4 * p4:4 * p4 + 4, 0:D]
        if on_act:
            nc.scalar.copy(dstc, st)
        else:
            nc.vector.tensor_copy(out=dstc, in_=st)
        for ti in range(4):
            t = p4 * 4 + ti
            for dc in range(2):
                nc.tensor.transpose(
                    pt[:, (ti * 2 + dc) * P:(ti * 2 + dc + 1) * P],
                    aug[:, t, dc * P:(dc + 1) * P], ident16,
                )
        dst = tr[:, :, p4 * 512:(p4 + 1) * 512]
        dstv = dst.rearrange("p two (t f) -> p t two f", t=4)
        srcv = pt.rearrange("p (t two f) -> p t two f", t=4, two=2)
        if on_act:
            nc.scalar.copy(dstv, srcv)
        else:
            nc.vector.tensor_copy(out=dstv, in_=srcv)

    # ---- S matmuls (fp8 DR) + exp + fused mask-select ----
    def s_half(t, H):
        ps = s_psum.tile([P, 1024], f32, tag="sp")
        for c in range(2):
            off = H * 1024 + c * 512
            nc.tensor.matmul(
                ps[:, c * 512:(c + 1) * 512],
                lhsT=q0t[:, :, t * P:(t + 1) * P],
                rhs=q1t[:, :, off:off + 512],
                start=True, stop=True, perf_mode=DR,
            )
        sl = slice(H * 1024, (H + 1) * 1024)
        nc.scalar.activation(out=e0[:, t, sl], in_=ps, func=EXP, scale=SCALE2)
        # masked entries -> exactly 0: e0 = min(wm1 + s0[p], e0)
        nc.vector.scalar_tensor_tensor(
            out=e0[:, t, sl], in0=wm1[:, sl], scalar=s0[:, t:t + 1],
            in1=e0[:, t, sl],
            op0=mybir.AluOpType.add, op1=mybir.AluOpType.min,
        )

    # ---- E^T stripe via DMA xbar transpose (sync DGE) ----
    def etr(t):
        nc.sync.dma_start(
            out=e1[:, :, t * P:(t + 1) * P], in_=e0[:, t, :], transpose=True
        )

    # ---- one pairwise-interleaved pair of out accumulation chains ----
    def out_pair(esrc, raug, odram, j0):
        pos = [o_psum.tile([P, NSUM], f32, tag="op", name=f"op{_k}") for _k in range(2)]
        for t in range(T):
            for k in range(2):
                j = j0 + k
                nc.tensor.matmul(
                    pos[k],
                    lhsT=esrc[:, t, j * P:(j + 1) * P],
                    rhs=raug[:, t, 0:NSUM],
                    start=(t == 0), stop=(t == T - 1),
                )
        ot = outp.tile([P, 2, D], f32, tag="ot")
        for k in range(2):
            rc = small.tile([P, 1], f32, tag="rc")
            nc.vector.reciprocal(rc, pos[k][:, D:D + 1])
            nc.scalar.mul(ot[:, k], pos[k][:, 0:D], rc)
        nc.gpsimd.dma_start(
            out=odram.rearrange("(j p) d -> p j d", p=P)[:, j0:j0 + 2, :], in_=ot
        )

    # ---- emission schedule ----
    prep_pack(q1, q1a, q1t, 0, on_act=True)
    prep_pack(q1, q1a, q1t, 1, on_act=False)
    prep_pack(q0, q0a, q0t, 0, on_act=True)
    for t in range(4):
        s_half(t, 0)
    prep_pack(q1, q1a, q1t, 2, on_act=True)
    prep_pack(q1, q1a, q1t, 3, on_act=False)
    prep_pack(q0, q0a, q0t, 1, on_act=True)
    for t in range(4, 8):
        s_half(t, 0)
    prep_pack(q0, q0a, q0t, 2, on_act=False)
    for t in range(8, 12):
        s_half(t, 0)
    prep_pack(q0, q0a, q0t, 3, on_act=True)
    for t in range(12, T):
        s_half(t, 0)
    for t in range(T):
        s_half(t, 1)
        etr(t)
        if t % 4 == 3:
            out_pair(e0, q0a, out1, (t // 4) * 2)
    out_pair(e0, q0a, out1, 8)
    out_pair(e1, q1a, out0, 0)
    out_pair(e0, q0a, out1, 10)
    out_pair(e1, q1a, out0, 2)
    out_pair(e0, q0a, out1, 12)
    out_pair(e1, q1a, out0, 4)
    out_pair(e0, q0a, out1, 14)
    out_pair(e1, q1a, out0, 6)
    for j0 in range(8, T, 2):
        out_pair(e1, q1a, out0, j0)


_CACHED_NC = None


def _build():
    global _CACHED_NC
    if _CACHED_NC is not None:
        return _CACHED_NC
    nc = bacc.Bacc("TRN2", target_bir_lowering=False, debug=False)
    io = {
        "q0": nc.dram_tensor("q0", [L, D], f32, kind="ExternalInput").ap(),
        "q1": nc.dram_tensor("q1", [L, D], f32, kind="ExternalInput").ap(),
        "mask0": nc.dram_tensor("mask0", [L], i32, kind="ExternalInput").ap(),
        "mask1": nc.dram_tensor("mask1", [L], i32, kind="ExternalInput").ap(),
        "out0": nc.dram_tensor("out0", [L, D], f32, kind="ExternalOutput").ap(),
        "out1": nc.dram_tensor("out1", [L, D], f32, kind="ExternalOutput").ap(),
    }
    with tile.TileContext(nc) as tc:
        with ExitStack() as ctx:
            _emit(tc, ctx, io)
    nc.compile()
    _CACHED_NC = nc
    return nc


def run_on_cores(q0, q1, mask0, mask1, trace=False):
    """Run the SPMD kernel; returns (out0, out1, BassKernelResults)."""
    nc = _build()
    in_maps = [
        {
            "q0": np.ascontiguousarray(q0[b], dtype=np.float32),
            "q1": np.ascontiguousarray(q1[b], dtype=np.float32),
            "mask0": np.ascontiguousarray(mask0[b], dtype=np.int32),
            "mask1": np.ascontiguousarray(mask1[b], dtype=np.int32),
        }
        for b in range(B)
    ]
    br = run_bass_kernel_spmd(nc, in_maps, list(range(B)), trace=trace)
    out0 = np.stack([br.results[b]["out0"] for b in range(B)])
    out1 = np.stack([br.results[b]["out1"] for b in range(B)])
    return out0, out1, br


def kernel(q0, q1, len0=None, len1=None, mask0=None, mask1=None, **_):
    q0 = np.asarray(q0, dtype=np.float32)
    q1 = np.asarray(q1, dtype=np.float32)
    mask0 = np.asarray(mask0, dtype=np.int32)
    mask1 = np.asarray(mask1, dtype=np.int32)
    out0, out1, _br = run_on_cores(q0, q1, mask0, mask1, trace=False)
    return out0, out1
